# revision 13
# baseline (speedup 1.0000x reference)
"""Trainium2 Bass kernel for nn_PostAttention (sparse_attention).

Computation (B=1, N=4096, H=8, d_qk=96, d_v=64):
    proj = qk @ W_qk -> q, k per head;  v = v_cls @ W_v per head
    S = q @ k.T * scale;  E = exp(S);  Z_i = sum_j E
    out_i = sum_j E_ij * m_ij * v_j / (Z_i * H * M_i),  M_i = sum_j m_ij

Sharding: 8 cores as 2 query-row groups x 4 head groups (2 heads/core).
Layout: everything computed transposed (S^T = [key j on partitions,
query i on free dim]) so that exp output E^T / P^T feed the P@V matmul
directly as the moving operand -- no on-chip transpose of the big
attention matrix.  Z and M column sums run as ones-vector matmuls packed
into spare PE column groups (tile_position).  The tiny output / scale
transposes use PE transpose-mode.

dtypes: projections and S matmul in float32r (tf32-like, 1 cyc/row);
E / mask / P / V in fp16 (DVE 2x mode, fp16 matmul 1 cyc/row);
all accumulation fp32 in PSUM.
"""
import os
import sys

sys.path.insert(0, "/opt/trn_rl_repo")
import numpy as np

import concourse.bass as bass
import concourse.mybir as mybir
import concourse.tile as tile
from concourse import bacc
from concourse.bass_utils import run_bass_kernel_spmd
from concourse.masks import make_identity

f32 = mybir.dt.float32
f32r = mybir.dt.float32r
f16 = mybir.dt.float16
FT = mybir.ActivationFunctionType
MUL = mybir.AluOpType.mult

N = 4096
H = 8
DQK = 96
DV = 64
R = 2              # row groups
C = 4              # head groups
HPC = H // C       # heads per core = 2
NQ = N // R        # queries per core = 2048
NIC = NQ // 512    # i-chunks per core = 4
NJT = N // 128     # j tiles = 32
SCALE = (256 // 8) ** -0.5
EXP_BIAS = -4.0    # uniform shift inside exp; cancels in the Z ratio

_CACHED = {}


def _build_nc():
    nc = bacc.Bacc(name="post_attention")

    qkT = nc.declare_dram_parameter("qkT", [768, N], f32r, isOutput=False)
    vT = nc.declare_dram_parameter("vT", [512, N], f32r, isOutput=False)
    wq = nc.declare_dram_parameter("wq", [768, HPC * DQK], f32r, isOutput=False)
    wk = nc.declare_dram_parameter("wk", [768, HPC * DQK], f32r, isOutput=False)
    wv = nc.declare_dram_parameter("wv", [512, HPC * DV], f32r, isOutput=False)
    maskT = nc.declare_dram_parameter("maskT", [N, NQ], f16, isOutput=False)
    out = nc.declare_dram_parameter("out", [NQ, HPC * DV], f32, isOutput=True)

    with tile.TileContext(nc) as tc:
        with (
            tc.tile_pool(name="const", bufs=1) as const,
            tc.tile_pool(name="persist", bufs=1) as persist,
            tc.tile_pool(name="rows", bufs=2) as rows,
        ):
            ones16 = const.tile([128, 1], f16)
            nc.vector.memset(ones16, 1.0)
            ident1 = const.tile([128, 1], f32)
            nc.vector.memset(ident1, 1.0)  # [p:p+1, 0:1] = 1x1 identity at any p
            ident = const.tile([128, 128], f32)
            make_identity(nc, ident)
            bias_t = const.tile([128, 1], f32)
            nc.vector.memset(bias_t, EXP_BIAS)

            QT = persist.tile([DQK, HPC, NQ], f32r)
            KT = persist.tile([DQK, HPC, N], f32r)
            V = persist.tile([128, NJT, HPC * DV], f16)

            # One compiled kernel serves all cores: the host rolls the N axis
            # of qkT / vT / mask columns so this core's query rows sit at
            # columns [0, NQ); the j-sum is order-invariant.

            # ---------------- projection phase ----------------
            with (
                tc.tile_pool(name="wpool", bufs=1) as wpool,
                tc.tile_pool(name="qs", bufs=3) as qs,
                tc.tile_pool(name="vtsb", bufs=1) as vtsb,
                tc.tile_pool(name="pp", bufs=1, space="PSUM") as pp,
                tc.tile_pool(name="ppv", bufs=1, space="PSUM") as ppv,
            ):
                wq_t = wpool.tile([128, 6, HPC * DQK], f32r)
                nc.sync.dma_start(out=wq_t, in_=wq.rearrange("(t p) n -> p t n", p=128))
                wk_t = wpool.tile([128, 6, HPC * DQK], f32r)
                nc.sync.dma_start(out=wk_t, in_=wk.rearrange("(t p) n -> p t n", p=128))
                wv_t = wpool.tile([128, 4, HPC * DV], f32r)
                nc.sync.dma_start(out=wv_t, in_=wv.rearrange("(t p) n -> p t n", p=128))

                VT_sb = vtsb.tile([128, N], f32)

                for n in range(8):
                    ncol = slice(n * 512, (n + 1) * 512)
                    qk_sl = qs.tile([128, 6, 512], f32r, tag="qksl")
                    nc.sync.dma_start(out=qk_sl, in_=qkT[:, ncol].rearrange("(t p) n -> p t n", p=128))
                    v_sl = qs.tile([128, 4, 512], f32r, tag="vsl")
                    nc.sync.dma_start(out=v_sl, in_=vT[:, ncol].rearrange("(t p) n -> p t n", p=128))

                    kt_ps = [pp.tile([DQK, 512], f32, tag=f"kt{h}", name=f"kt_ps{h}") for h in range(HPC)]
                    for h in range(HPC):
                        for c in range(6):
                            nc.tensor.matmul(
                                kt_ps[h],
                                lhsT=wk_t[:, c, h * DQK : (h + 1) * DQK],
                                rhs=qk_sl[:, c, :],
                                start=(c == 0),
                                stop=(c == 5),
                            )
                        nc.vector.tensor_copy(KT[:, h, ncol], kt_ps[h])

                    if n < NIC:  # query rows live in columns [0, NQ) after host roll
                        qt_ps = [pp.tile([DQK, 512], f32, tag=f"qt{h}", name=f"qt_ps{h}") for h in range(HPC)]
                        for h in range(HPC):
                            for c in range(6):
                                nc.tensor.matmul(
                                    qt_ps[h],
                                    lhsT=wq_t[:, c, h * DQK : (h + 1) * DQK],
                                    rhs=qk_sl[:, c, :],
                                    start=(c == 0),
                                    stop=(c == 5),
                                )
                            nc.vector.tensor_copy(QT[:, h, ncol], qt_ps[h])

                    vt_ps = ppv.tile([128, 512], f32, tag="vt")
                    for c in range(4):
                        nc.tensor.matmul(
                            vt_ps,
                            lhsT=wv_t[:, c, :],
                            rhs=v_sl[:, c, :],
                            start=(c == 0),
                            stop=(c == 3),
                        )
                    nc.vector.tensor_copy(VT_sb[:, ncol], vt_ps)

                # transpose VT [e', N] -> V [j, e'] tiles (fp16)
                for jt in range(NJT):
                    vtr = ppv.tile([128, 128], f32, tag="vtr")
                    nc.tensor.transpose(vtr, VT_sb[:, jt * 128 : (jt + 1) * 128], ident)
                    nc.vector.tensor_copy(V[:, jt, :], vtr)

            # ---------------- attention phase ----------------
            with (
                tc.tile_pool(name="mt", bufs=3) as mtp,
                tc.tile_pool(name="ep", bufs=3) as ep,
                tc.tile_pool(name="fin", bufs=2) as fin,
                tc.tile_pool(name="ps_s", bufs=2, space="PSUM") as ps_s,
                tc.tile_pool(name="ps_o", bufs=1, space="PSUM") as ps_o,
                tc.tile_pool(name="ps_zm", bufs=1, space="PSUM") as ps_zm,
                tc.tile_pool(name="ps_f", bufs=1, space="PSUM") as ps_f,
            ):
                for ic in range(NIC):
                    icol = slice(ic * 512, (ic + 1) * 512)
                    # each accumulation stream gets its own PSUM bank: the
                    # first matmul of a group (start=True) clears has_written
                    # for the WHOLE bank on HW, so streams must not share one.
                    o_ps = [ps_o.tile([128, 512], f32, tag=f"o{h}", name=f"o_ps{h}") for h in range(HPC)]
                    zm_ps = [ps_zm.tile([128, 512], f32, tag=f"zm{k}", name=f"zm_ps{k}") for k in range(3)]
                    for jt in range(NJT):
                        jrow = slice(jt * 128, (jt + 1) * 128)
                        mt = mtp.tile([128, 512], f16, tag="mt")
                        nc.sync.dma_start(out=mt, in_=maskT[jrow, icol])
                        # M column sums -> zm_ps partition 64 (col group 2)
                        nc.tensor.matmul(
                            zm_ps[2][64:65, :], lhsT=ones16, rhs=mt,
                            start=(jt == 0), stop=(jt == NJT - 1),
                            tile_position=(0, 64), skip_group_check=True,
                        )
                        for h in range(HPC):
                            s_ps = ps_s.tile([128, 512], f32, tag="s")
                            nc.tensor.matmul(
                                s_ps,
                                lhsT=KT[:, h, jrow],
                                rhs=QT[:, h, icol],
                            )
                            e_t = ep.tile([128, 512], f16, tag="e")
                            nc.scalar.activation(e_t, s_ps, FT.Exp, bias=bias_t, scale=SCALE)
                            # Z_h column sums -> zm_ps partition 32*h
                            nc.tensor.matmul(
                                zm_ps[h][32 * h : 32 * h + 1, :], lhsT=ones16, rhs=e_t,
                                start=(jt == 0), stop=(jt == NJT - 1),
                                tile_position=(0, 32 * h), skip_group_check=True,
                            )
                            p_t = ep.tile([128, 512], f16, tag="p")
                            nc.vector.tensor_mul(p_t, e_t, mt)
                            nc.tensor.matmul(
                                o_ps[h][64 * h : 64 * (h + 1), :],
                                lhsT=V[:, jt, 64 * h : 64 * (h + 1)],
                                rhs=p_t,
                                start=(jt == 0), stop=(jt == NJT - 1),
                                tile_position=(0, 64 * h), skip_group_check=True,
                            )
                    # ---- finalize i-chunk ----
                    # rows of zm_ps (z0 @p0, z1 @p32, m @p64) -> SBUF at the
                    # same partitions, then PE-transpose each 128-query chunk
                    # into columns; all per-partition math happens in column
                    # form at base partition 0.
                    zm_sb = fin.tile([65, 512], f32, tag="zmsb")
                    for k in range(3):
                        p0 = 32 * k
                        nc.vector.tensor_copy(zm_sb[p0 : p0 + 1, :], zm_ps[k][p0 : p0 + 1, :])
                    zmT_ps = ps_f.tile([128, 12], f32, tag="fps")
                    for k in range(3):
                        p0 = 32 * k
                        for q in range(4):
                            nc.tensor.transpose(
                                zmT_ps[:, k * 4 + q : k * 4 + q + 1],
                                zm_sb[p0 : p0 + 1, q * 128 : (q + 1) * 128],
                                ident1[p0 : p0 + 1, :],
                            )
                    zmT_sb = fin.tile([128, 12], f32, tag="zmTsb")
                    nc.vector.tensor_copy(zmT_sb, zmT_ps)
                    wtmp = fin.tile([128, HPC * 4], f32, tag="wtmp")
                    w_col = fin.tile([128, HPC * 4], f32, tag="wcol")
                    for h in range(HPC):
                        nc.vector.tensor_mul(wtmp[:, h * 4 : (h + 1) * 4],
                                             zmT_sb[:, h * 4 : (h + 1) * 4],
                                             zmT_sb[:, 8:12])
                    nc.vector.reciprocal(w_col, wtmp)

                    o_sb = fin.tile([128, 512], f32, tag="osb")
                    for h in range(HPC):
                        nc.vector.tensor_copy(o_sb[64 * h : 64 * (h + 1), :],
                                              o_ps[h][64 * h : 64 * (h + 1), :])
                    fin_sb = fin.tile([128, 4, HPC, DV], f32, tag="finsb")
                    for h in range(HPC):
                        for q in range(4):
                            ot_ps = ps_f.tile([128, DV], f32, tag="fps")
                            nc.tensor.transpose(
                                ot_ps,
                                o_sb[64 * h : 64 * (h + 1), q * 128 : (q + 1) * 128],
                                ident[64 * h : 64 * h + DV, 64 * h : 64 * h + DV],
                            )
                            nc.vector.tensor_scalar_mul(
                                fin_sb[:, q, h, :], ot_ps,
                                w_col[:, h * 4 + q : h * 4 + q + 1],
                            )
                    nc.sync.dma_start(
                        out=out[icol, :].rearrange("(q p) (h e) -> p q h e", p=128, h=HPC),
                        in_=fin_sb,
                    )

    nc.finalize()
    return nc


def kernel(**inputs) -> np.ndarray:
    qk = np.asarray(inputs["qk"], dtype=np.float32)        # [1, N, 768]
    v_cls = np.asarray(inputs["v_cls"], dtype=np.float32)  # [1, N, 512]
    masks = np.asarray(inputs["masks"], dtype=np.float32)  # [1, N, N]
    W_qk = np.asarray(inputs["W_qk"], dtype=np.float32)    # [768, 1536]
    W_v = np.asarray(inputs["W_v"], dtype=np.float32)      # [512, 512]

    if "nc" not in _CACHED:
        _CACHED["nc"] = _build_nc()
    nc = _CACHED["nc"]

    mask0 = masks[0].astype(np.float16)
    # Roll the key/value axis per row group so each core's query rows start at
    # column 0; the kernel reads Q from columns [0, NQ) and pairs KT j-tiles
    # with identically rolled mask columns, so the j-sum is just reordered.
    qkT_rg, vT_rg, mask_rg = [], [], []
    for rg in range(R):
        h0 = rg * NQ
        qk_roll = np.roll(qk[0], -h0, axis=0)
        v_roll = np.roll(v_cls[0], -h0, axis=0)
        qkT_rg.append(np.ascontiguousarray(qk_roll.T))      # [768, N]
        vT_rg.append(np.ascontiguousarray(v_roll.T))        # [512, N]
        mask_rg.append(np.ascontiguousarray(np.roll(mask0[h0 : h0 + NQ], -h0, axis=1).T))
    wq_hg, wk_hg, wv_hg = [], [], []
    for hg in range(C):
        hs = hg * HPC
        wq_hg.append(np.ascontiguousarray(W_qk[:, hs * DQK : (hs + HPC) * DQK]))
        wk_hg.append(np.ascontiguousarray(W_qk[:, 768 + hs * DQK : 768 + (hs + HPC) * DQK]))
        wv_hg.append(np.ascontiguousarray(W_v[:, hs * DV : (hs + HPC) * DV] / H))
    in_maps = []
    for core in range(8):
        rg, hg = divmod(core, C)
        in_maps.append({
            "qkT": qkT_rg[rg],
            "vT": vT_rg[rg],
            "wq": wq_hg[hg],
            "wk": wk_hg[hg],
            "wv": wv_hg[hg],
            "maskT": mask_rg[rg],
        })

    trace = os.environ.get("KERNEL_TRACE", "0") == "1"
    res = run_bass_kernel_spmd(nc, in_maps, list(range(8)), trace=trace)
    if trace:
        _CACHED["exec_time_ns"] = res.exec_time_ns
        _CACHED["mean_exec_time_ns"] = res.mean_exec_time_ns

    out = np.empty((1, N, 512), dtype=np.float32)
    for core in range(8):
        rg, hg = divmod(core, C)
        out[0, rg * NQ : (rg + 1) * NQ, hg * HPC * DV : (hg + 1) * HPC * DV] = res.results[core]["out"]
    return out
